# revision 30
# baseline (speedup 1.0000x reference)
"""Distributed causal RoPE attention for Trainium2 (8 NeuronCores).

Problem: nn_CausalRpeAttn — B=2, S=2048, D=1024, H=16, Dh=64, fp32.

Sharding (data + head parallel): core c handles batch c//4 and heads
4*(c%4) .. 4*(c%4)+3 (a 256-wide feature slice). Wq/Wk/Wv are split
column-wise (by output head group), Wo row-wise. Each core writes its
full [1024, 2048] (transposed) partial output projection (with bo/4
pre-added); the host unshards by summing the 4 partials per batch and
transposing back. Attention itself is fully independent per
(batch, head), so the only cross-core combination is that final sum.

On-device layout notes:
 - Everything is kept feature-major ("transposed", [feat, seq]) so all
   matmuls have moving dim 512.
 - QKV projections run in bf16 (inputs pre-cast on host); q/k after
   RoPE are fp32r (full PE rate at moving dim >=256, ~1e-4 rounding).
 - Scores are computed transposed: sT[k_pos, q] per head, two heads per
   128-partition group row-packed on the PE (Dh=64 contraction).
 - Softmax skips the max subtraction (scores/8 are O(5), exp is safe in
   fp32) so no cross-partition max is needed. exp runs on the Scalar
   engine straight out of PSUM with the 1/8 scale folded in, writing
   bf16 probs trimmed to the causal width. Diagonal tiles are masked by
   gpsimd affine_select.
 - v gets an appended ones-row so the PV matmul (lhsT=[128,65]) yields
   both the weighted sum and the softmax denominator in one PSUM tile.
 - RoPE's rotate-half partition swap is done by four SBUF->SBUF DMAs
   on full-width [32, 2048] blocks so the vector engine only runs two
   full-width multiplies (cos/sin); the bias-add runs on the Scalar
   engine and the final add on GpSimd.
 - The Wo partial for q tile t is emitted one tile late so the PE never
   waits on the reciprocal->broadcast->normalize chain.
"""

import os
import ml_dtypes
import numpy as np

B, S, D, H, DH = 2, 2048, 1024, 16, 64
N_CORES = 8
FPC = 256  # features per core (4 heads)
QT = 512
NQT = S // QT  # 4
NST = S // 512  # 4 s-tiles for projections

_cache = {}
last_run_info = {}


def _build():
    import concourse.bass as bass
    import concourse.mybir as mybir
    import concourse.tile as tile
    from concourse import bacc
    from concourse.masks import make_identity

    F32 = mybir.dt.float32
    F32R = mybir.dt.float32r
    BF16 = mybir.dt.bfloat16
    AOP = mybir.AluOpType
    EXP = mybir.ActivationFunctionType.Exp

    nc = bacc.Bacc("TRN2", target_bir_lowering=False, debug=False,
                   num_devices=N_CORES)

    qkvT_e = nc.dram_tensor("qkvT", [D, S], BF16, kind="ExternalInput").ap()
    wq_e = nc.dram_tensor("wq", [D, FPC], BF16, kind="ExternalInput").ap()
    wk_e = nc.dram_tensor("wk", [D, FPC], BF16, kind="ExternalInput").ap()
    wv_e = nc.dram_tensor("wv", [D, FPC], BF16, kind="ExternalInput").ap()
    wo_e = nc.dram_tensor("wo", [FPC, D], F32R, kind="ExternalInput").ap()
    bq_e = nc.dram_tensor("bq", [FPC], F32, kind="ExternalInput").ap()
    bk_e = nc.dram_tensor("bk", [FPC], F32, kind="ExternalInput").ap()
    bv_e = nc.dram_tensor("bv", [FPC], F32, kind="ExternalInput").ap()
    bo_e = nc.dram_tensor("bo", [D], F32, kind="ExternalInput").ap()
    cos2_e = nc.dram_tensor("cos2", [128, S], F32, kind="ExternalInput").ap()
    sinx_e = nc.dram_tensor("sinx", [128, S], F32, kind="ExternalInput").ap()
    out_e = nc.dram_tensor("out", [D, S], F32, kind="ExternalOutput").ap()

    from contextlib import ExitStack
    with tile.TileContext(nc) as tc:
        with ExitStack() as ctx:
            ep = ctx.enter_context
            consts = ep(tc.tile_pool(name="consts", bufs=1))
            xin_pool = ep(tc.tile_pool(name="xin", bufs=1))
            rope_pool = ep(tc.tile_pool(name="rope", bufs=4))
            tmp_pool = ep(tc.tile_pool(name="tmp", bufs=2))
            qb_pool = ep(tc.tile_pool(name="qb", bufs=2))
            qbs_pool = ep(tc.tile_pool(name="qbs", bufs=2))
            vtmp_pool = ep(tc.tile_pool(name="vtmp", bufs=2))
            vsb_pool = ep(tc.tile_pool(name="vsb", bufs=1))
            probs_pool = ep(tc.tile_pool(name="probs", bufs=6))
            woin_pool = ep(tc.tile_pool(name="woin", bufs=2))
            rb_pool = ep(tc.tile_pool(name="rb", bufs=2))
            rec_pool = ep(tc.tile_pool(name="rec", bufs=2))
            osb_pool = ep(tc.tile_pool(name="osb", bufs=3))
            ps_pool = ep(tc.tile_pool(name="ps", bufs=4, space="PSUM"))
            big_pool = ep(tc.tile_pool(name="big", bufs=2, space="PSUM"))
            dram_pool = ep(tc.tile_pool(name="dram", bufs=1, space="DRAM"))
            # ---- constants (wq + x chunks first so compute starts early) ----
            wq_sb = consts.tile([128, 8, FPC], BF16, tag="wq")
            nc.sync.dma_start(out=wq_sb[:],
                              in_=wq_e.rearrange("(kt p) f -> p kt f", p=128))
            b_sbs = []
            for name, be in (("bq", bq_e), ("bk", bk_e), ("bv", bv_e)):
                t = consts.tile([128, 2], F32, tag=name)
                nc.sync.dma_start(out=t[:], in_=be.rearrange("(t p) -> p t", p=128))
                b_sbs.append(t)
            bq_sb, bk_sb, bv_sb = b_sbs
            bo_sb = consts.tile([128, 8], F32, tag="bo")
            nc.sync.dma_start(out=bo_sb[:], in_=bo_e.rearrange("(t p) -> p t", p=128))
            cos2_sb = consts.tile([128, S], F32, tag="cos2")
            nc.sync.dma_start(out=cos2_sb[:], in_=cos2_e)
            sinx_sb = consts.tile([128, S], F32, tag="sinx")
            nc.sync.dma_start(out=sinx_sb[:], in_=sinx_e)

            x_all = xin_pool.tile([128, 8, S], BF16, tag="x")
            for st in range(NST):
                for kt in range(8):
                    nc.sync.dma_start(
                        out=x_all[:, kt, st * 512:(st + 1) * 512],
                        in_=qkvT_e[kt * 128:(kt + 1) * 128,
                                   st * 512:(st + 1) * 512])

            wk_sb = consts.tile([128, 8, FPC], BF16, tag="wk")
            wv_sb = consts.tile([128, 8, FPC], BF16, tag="wv")
            for t, we in ((wk_sb, wk_e), (wv_sb, wv_e)):
                nc.sync.dma_start(out=t[:],
                                  in_=we.rearrange("(kt p) f -> p kt f", p=128))
            wo_sb = consts.tile([128, 2, D], F32R, tag="wo")
            nc.sync.dma_start(out=wo_sb[:],
                              in_=wo_e.rearrange("(pt p) f -> p pt f", p=128))
            ident = consts.tile([128, 128], F32, tag="ident")
            make_identity(nc, ident[:])


            # rope targets: [feat 128, S] per partition-tile, fp32r
            qrot = [rope_pool.tile([128, S], BF16, tag="rope", name=f"qrot{i}")
                    for i in range(2)]
            krot = [rope_pool.tile([128, S], BF16, tag="rope", name=f"krot{i}")
                    for i in range(2)]
            # v with ones row: [s_tile part, 16 s-tiles, 4 heads, 64+1]
            v_sb = vsb_pool.tile([128, 16, 4, DH + 1], BF16, tag="v")
            nc.vector.memset(
                v_sb[:].rearrange("p a b c -> p (a b c)"), 1.0)

            def rope_big(qb, dst):
                # qb [128, S] holds bias-added projection; swap halves via DMA
                qbs = qbs_pool.tile([128, S], F32, tag="qbs")
                for h in (0, 1):
                    p0 = h * 64
                    nc.sync.dma_start(out=qbs[p0:p0 + 32, :],
                                      in_=qb[p0 + 32:p0 + 64, :])
                    nc.sync.dma_start(out=qbs[p0 + 32:p0 + 64, :],
                                      in_=qb[p0:p0 + 32, :])
                nc.vector.tensor_mul(out=dst[:], in0=qb[:], in1=cos2_sb[:])
                tmp = tmp_pool.tile([128, S], F32R, tag="tmp")
                nc.vector.tensor_mul(out=tmp[:], in0=qbs[:], in1=sinx_sb[:])
                nc.gpsimd.tensor_add(out=dst[:], in0=dst[:], in1=tmp[:])

            # ---- phase A: projections (proj-outer) + RoPE + v transpose ----
            for proj in range(3):
                w_sb = (wq_sb, wk_sb, wv_sb)[proj]
                b_sb = b_sbs[proj]
                for pt in range(2):
                    qb = None
                    if proj < 2:
                        qb = qb_pool.tile([128, S], F32, tag="qb")
                    for st in range(NST):
                        ss = slice(st * 512, (st + 1) * 512)
                        ps = ps_pool.tile([128, 512], F32, tag="ps")
                        for kt in range(8):
                            nc.tensor.matmul(
                                ps[:], w_sb[:, kt, pt * 128:(pt + 1) * 128],
                                x_all[:, kt, ss],
                                start=(kt == 0), stop=(kt == 7))
                        if proj < 2:
                            nc.scalar.activation(
                                out=qb[:, ss], in_=ps[:],
                                func=mybir.ActivationFunctionType.Identity,
                                bias=b_sb[:, pt:pt + 1])
                        else:
                            vt = vtmp_pool.tile([128, 512], F32, tag="vt")
                            nc.scalar.activation(
                                out=vt[:], in_=ps[:],
                                func=mybir.ActivationFunctionType.Identity,
                                bias=bv_sb[:, pt:pt + 1])
                            for j in range(4):
                                sti = st * 4 + j
                                ps_tr = big_pool.tile([128, 128], F32,
                                                      tag="big")
                                nc.tensor.transpose(
                                    ps_tr[:], vt[:, j * 128:(j + 1) * 128],
                                    ident[:])
                                nc.vector.tensor_copy(
                                    out=v_sb[:, sti, 2 * pt:2 * pt + 2, 0:DH],
                                    in_=ps_tr[:].rearrange(
                                        "p (h d) -> p h d", h=2))
                    if proj == 0:
                        rope_big(qb, qrot[pt])
                    elif proj == 1:
                        rope_big(qb, krot[pt])

            # ---- phase B: attention (qt-outer; per-qt Wo partial +
            # chunked bf16 ReduceScatter overlapping later q tiles) ----
            woin = [woin_pool.tile([128, S], F32R, tag="woin", name=f"woin{i}")
                    for i in range(2)]
            def scores(kt, qt, pair, qsl):
                ksl = slice(kt * 128, (kt + 1) * 128)
                ps_s = big_pool.tile([128, 1024], F32, tag="big",
                                     name="ps_s")
                nc.tensor.matmul(
                    ps_s[:, 0:512], krot[pair][0:64, ksl],
                    qrot[pair][0:64, qsl], start=True, stop=True)
                nc.tensor.matmul(
                    ps_s[:, 512:1024], krot[pair][64:128, ksl],
                    qrot[pair][64:128, qsl], start=True, stop=True)
                pr = probs_pool.tile([128, 1024], BF16, tag="pr", name="pr")
                psv = ps_s[:].rearrange("p (h q) -> p h q", h=2)
                prv = pr[:].rearrange("p (h q) -> p h q", h=2)
                off = kt * 128 - qt * 512
                if off <= 0:
                    nc.scalar.activation(out=pr[:], in_=ps_s[:],
                                         func=EXP, scale=0.125)
                else:
                    nc.scalar.activation(out=prv[:, :, off:512],
                                         in_=psv[:, :, off:512],
                                         func=EXP, scale=0.125)
                if off >= 0:
                    nc.gpsimd.affine_select(
                        out=prv[:, :, off:off + 128],
                        in_=prv[:, :, off:off + 128],
                        pattern=[[0, 2], [1, 128]],
                        compare_op=AOP.is_ge, fill=0.0,
                        base=0,
                        channel_multiplier=-1)
                return pr

            def pv(kt, pr, pv_a, pv_b, h0, nkt, qt):
                off = max(0, kt * 128 - qt * 512)
                nc.tensor.matmul(
                    pv_a[:, off:512], v_sb[:, kt, h0, :],
                    pr[:, off:512],
                    start=(kt == 0), stop=(kt == nkt - 1))
                nc.tensor.matmul(
                    pv_b[:, off:512], v_sb[:, kt, h0 + 1, :],
                    pr[:, 512 + off:1024],
                    start=(kt == 0), stop=(kt == nkt - 1))

            def wo_block(qt):
                qsl = slice(qt * 512, (qt + 1) * 512)
                for dm in range(8):
                    ps_o = ps_pool.tile([128, 512], F32, tag="ps",
                                        name="ps_o")
                    for pt in range(2):
                        nc.tensor.matmul(
                            ps_o[:], wo_sb[:, pt, dm * 128:(dm + 1) * 128],
                            woin[pt][:, qsl], start=(pt == 0), stop=(pt == 1))
                    ot = osb_pool.tile([128, QT], F32, tag="ot")
                    if dm % 2 == 0:
                        nc.vector.tensor_scalar_add(out=ot[:], in0=ps_o[:],
                                                    scalar1=bo_sb[:, dm:dm + 1])
                    else:
                        nc.scalar.activation(
                            out=ot[:], in_=ps_o[:],
                            func=mybir.ActivationFunctionType.Identity,
                            bias=bo_sb[:, dm:dm + 1])
                    nc.sync.dma_start(
                        out=out_e[dm * 128:(dm + 1) * 128, qsl], in_=ot[:])

            for qt in range(NQT):
                qsl = slice(qt * 512, (qt + 1) * 512)
                for pair in range(2):
                    pv_a = ps_pool.tile([DH + 1, 512], F32, tag="ps",
                                        name="pv_a")
                    pv_b = ps_pool.tile([DH + 1, 512], F32, tag="ps",
                                        name="pv_b")
                    nkt = 4 * qt + 4
                    h0 = 2 * pair

                    # software pipeline: scores run one kt ahead of pv
                    pr_prev = scores(0, qt, pair, qsl)
                    for kt in range(1, nkt):
                        pr_k = scores(kt, qt, pair, qsl)
                        pv(kt - 1, pr_prev, pv_a, pv_b, h0, nkt, qt)
                        pr_prev = pr_k
                    pv(nkt - 1, pr_prev, pv_a, pv_b, h0, nkt, qt)

                    rec = rec_pool.tile([1, 1024], F32, tag="rec")
                    nc.vector.reciprocal(out=rec[0:1, 0:512],
                                         in_=pv_a[DH:DH + 1, :])
                    nc.vector.reciprocal(out=rec[0:1, 512:1024],
                                         in_=pv_b[DH:DH + 1, :])
                    rb = rb_pool.tile([128, 1024], F32, tag="rb")
                    nc.gpsimd.partition_broadcast(rb[:], rec[0:1, :])
                    nc.vector.tensor_mul(out=woin[pair][0:64, qsl],
                                         in0=pv_a[0:DH, :],
                                         in1=rb[0:64, 0:512])
                    nc.vector.tensor_mul(out=woin[pair][64:128, qsl],
                                         in0=pv_b[0:DH, :],
                                         in1=rb[64:128, 512:1024])
                if qt > 0:
                    wo_block(qt - 1)
            wo_block(NQT - 1)


    nc.compile()
    return nc


def kernel(qkv, cos, sin, Wq, bq, Wk, bk, Wv, bv, Wo, bo):
    from concourse.bass_utils import run_bass_kernel_spmd

    qkv = np.asarray(qkv, dtype=np.float32)
    cos = np.asarray(cos, dtype=np.float32)
    sin = np.asarray(sin, dtype=np.float32)
    Wq, bq = np.asarray(Wq, np.float32), np.asarray(bq, np.float32)
    Wk, bk = np.asarray(Wk, np.float32), np.asarray(bk, np.float32)
    Wv, bv = np.asarray(Wv, np.float32), np.asarray(bv, np.float32)
    Wo, bo = np.asarray(Wo, np.float32), np.asarray(bo, np.float32)

    if "nc" not in _cache:
        _cache["nc"] = _build()
    nc = _cache["nc"]

    cos2 = np.ascontiguousarray(np.tile(cos.T, (2, 1)))  # [128, S]
    sinx = np.tile(sin.T, (2, 1)).copy()
    sinx[0:32] *= -1.0
    sinx[64:96] *= -1.0
    sinx = np.ascontiguousarray(sinx)

    bf = ml_dtypes.bfloat16
    bo4 = np.ascontiguousarray(bo * 0.25)
    in_maps = []
    for c in range(N_CORES):
        b, g = c // 4, c % 4
        hsl = slice(g * FPC, (g + 1) * FPC)
        in_maps.append({
            "qkvT": np.ascontiguousarray(qkv[b].T.astype(bf)),
            "wq": np.ascontiguousarray(Wq[hsl, :].T.astype(bf)),
            "wk": np.ascontiguousarray(Wk[hsl, :].T.astype(bf)),
            "wv": np.ascontiguousarray(Wv[hsl, :].T.astype(bf)),
            "wo": np.ascontiguousarray(Wo[:, hsl].T),
            "bq": np.ascontiguousarray(bq[hsl]),
            "bk": np.ascontiguousarray(bk[hsl]),
            "bv": np.ascontiguousarray(bv[hsl]),
            "bo": bo4,
            "cos2": cos2,
            "sinx": sinx,
        })

    trace = bool(os.environ.get("KERNEL_TRACE"))
    res = run_bass_kernel_spmd(nc, in_maps, list(range(N_CORES)), trace=trace)
    last_run_info["exec_time_ns"] = res.exec_time_ns
    last_run_info["results"] = res

    out = np.empty((B, S, D), dtype=np.float32)
    for b in range(B):
        oT = (res.results[4 * b]["out"] + res.results[4 * b + 1]["out"]
              + res.results[4 * b + 2]["out"] + res.results[4 * b + 3]["out"])
        out[b] = oT.T
    return out


# revision 31
# speedup vs baseline: 1.0319x; 1.0319x over previous
"""Distributed causal RoPE attention for Trainium2 (8 NeuronCores).

Problem: nn_CausalRpeAttn — B=2, S=2048, D=1024, H=16, Dh=64, fp32.

Sharding (data + head parallel): core c handles batch c//4 and heads
4*(c%4) .. 4*(c%4)+3 (a 256-wide feature slice). Wq/Wk/Wv are split
column-wise (by output head group), Wo row-wise. Each core writes its
full [1024, 2048] (transposed) partial output projection (with bo/4
pre-added); the host unshards by summing the 4 partials per batch and
transposing back. Attention itself is fully independent per
(batch, head), so the only cross-core combination is that final sum.

On-device layout notes:
 - Everything is kept feature-major ("transposed", [feat, seq]) so all
   matmuls have moving dim 512.
 - QKV projections run in bf16 (inputs pre-cast on host); q/k after
   RoPE are fp32r (full PE rate at moving dim >=256, ~1e-4 rounding).
 - Scores are computed transposed: sT[k_pos, q] per head, two heads per
   128-partition group row-packed on the PE (Dh=64 contraction).
 - Softmax skips the max subtraction (scores/8 are O(5), exp is safe in
   fp32) so no cross-partition max is needed. exp runs on the Scalar
   engine straight out of PSUM with the 1/8 scale folded in, writing
   bf16 probs trimmed to the causal width. Diagonal tiles are masked by
   gpsimd affine_select.
 - v gets an appended ones-row so the PV matmul (lhsT=[128,65]) yields
   both the weighted sum and the softmax denominator in one PSUM tile.
 - RoPE's rotate-half partition swap is done by four SBUF->SBUF DMAs
   on full-width [32, 2048] blocks so the vector engine only runs two
   full-width multiplies (cos/sin); the bias-add runs on the Scalar
   engine and the final add on GpSimd.
 - The Wo partial for q tile t is emitted one tile late so the PE never
   waits on the reciprocal->broadcast->normalize chain.
"""

import os
import ml_dtypes
import numpy as np

B, S, D, H, DH = 2, 2048, 1024, 16, 64
N_CORES = 8
FPC = 256  # features per core (4 heads)
QT = 512
NQT = S // QT  # 4
NST = S // 512  # 4 s-tiles for projections

_cache = {}
last_run_info = {}


def _build():
    import concourse.bass as bass
    import concourse.mybir as mybir
    import concourse.tile as tile
    from concourse import bacc
    from concourse.masks import make_identity

    F32 = mybir.dt.float32
    F32R = mybir.dt.float32r
    BF16 = mybir.dt.bfloat16
    AOP = mybir.AluOpType
    EXP = mybir.ActivationFunctionType.Exp

    nc = bacc.Bacc("TRN2", target_bir_lowering=False, debug=False,
                   num_devices=N_CORES)

    qkvT_e = nc.dram_tensor("qkvT", [D, S], BF16, kind="ExternalInput").ap()
    wq_e = nc.dram_tensor("wq", [D, FPC], BF16, kind="ExternalInput").ap()
    wk_e = nc.dram_tensor("wk", [D, FPC], BF16, kind="ExternalInput").ap()
    wv_e = nc.dram_tensor("wv", [D, FPC], BF16, kind="ExternalInput").ap()
    wo_e = nc.dram_tensor("wo", [FPC, D], F32R, kind="ExternalInput").ap()
    bq_e = nc.dram_tensor("bq", [FPC], F32, kind="ExternalInput").ap()
    bk_e = nc.dram_tensor("bk", [FPC], F32, kind="ExternalInput").ap()
    bv_e = nc.dram_tensor("bv", [FPC], F32, kind="ExternalInput").ap()
    bo_e = nc.dram_tensor("bo", [D], F32, kind="ExternalInput").ap()
    cos2_e = nc.dram_tensor("cos2", [128, S], F32, kind="ExternalInput").ap()
    sinx_e = nc.dram_tensor("sinx", [128, S], F32, kind="ExternalInput").ap()
    out_e = nc.dram_tensor("out", [D, S], F32, kind="ExternalOutput").ap()

    from contextlib import ExitStack
    with tile.TileContext(nc) as tc:
        with ExitStack() as ctx:
            ep = ctx.enter_context
            consts = ep(tc.tile_pool(name="consts", bufs=1))
            xin_pool = ep(tc.tile_pool(name="xin", bufs=1))
            rope_pool = ep(tc.tile_pool(name="rope", bufs=4))
            tmp_pool = ep(tc.tile_pool(name="tmp", bufs=2))
            qb_pool = ep(tc.tile_pool(name="qb", bufs=2))
            qbs_pool = ep(tc.tile_pool(name="qbs", bufs=2))
            vtmp_pool = ep(tc.tile_pool(name="vtmp", bufs=2))
            vsb_pool = ep(tc.tile_pool(name="vsb", bufs=1))
            probs_pool = ep(tc.tile_pool(name="probs", bufs=4))
            woin_pool = ep(tc.tile_pool(name="woin", bufs=2))
            rb_pool = ep(tc.tile_pool(name="rb", bufs=2))
            rec_pool = ep(tc.tile_pool(name="rec", bufs=2))
            osb_pool = ep(tc.tile_pool(name="osb", bufs=3))
            ps_pool = ep(tc.tile_pool(name="ps", bufs=4, space="PSUM"))
            big_pool = ep(tc.tile_pool(name="big", bufs=2, space="PSUM"))
            dram_pool = ep(tc.tile_pool(name="dram", bufs=1, space="DRAM"))
            # ---- constants (wq + x chunks first so compute starts early) ----
            wq_sb = consts.tile([128, 8, FPC], BF16, tag="wq")
            nc.sync.dma_start(out=wq_sb[:],
                              in_=wq_e.rearrange("(kt p) f -> p kt f", p=128))
            b_sbs = []
            for name, be in (("bq", bq_e), ("bk", bk_e), ("bv", bv_e)):
                t = consts.tile([128, 2], F32, tag=name)
                nc.sync.dma_start(out=t[:], in_=be.rearrange("(t p) -> p t", p=128))
                b_sbs.append(t)
            bq_sb, bk_sb, bv_sb = b_sbs
            bo_sb = consts.tile([128, 8], F32, tag="bo")
            nc.sync.dma_start(out=bo_sb[:], in_=bo_e.rearrange("(t p) -> p t", p=128))
            cos2_sb = consts.tile([128, S], F32, tag="cos2")
            nc.sync.dma_start(out=cos2_sb[:], in_=cos2_e)
            sinx_sb = consts.tile([128, S], F32, tag="sinx")
            nc.sync.dma_start(out=sinx_sb[:], in_=sinx_e)

            x_all = xin_pool.tile([128, 8, S], BF16, tag="x")
            for kt in range(8):
                nc.sync.dma_start(out=x_all[:, kt, :],
                                  in_=qkvT_e[kt * 128:(kt + 1) * 128, :])

            wk_sb = consts.tile([128, 8, FPC], BF16, tag="wk")
            wv_sb = consts.tile([128, 8, FPC], BF16, tag="wv")
            for t, we in ((wk_sb, wk_e), (wv_sb, wv_e)):
                nc.sync.dma_start(out=t[:],
                                  in_=we.rearrange("(kt p) f -> p kt f", p=128))
            wo_sb = consts.tile([128, 2, D], F32R, tag="wo")
            nc.sync.dma_start(out=wo_sb[:],
                              in_=wo_e.rearrange("(pt p) f -> p pt f", p=128))
            ident = consts.tile([128, 128], F32, tag="ident")
            make_identity(nc, ident[:])


            # rope targets: [feat 128, S] per partition-tile, fp32r
            qrot = [rope_pool.tile([128, S], BF16, tag="rope", name=f"qrot{i}")
                    for i in range(2)]
            krot = [rope_pool.tile([128, S], BF16, tag="rope", name=f"krot{i}")
                    for i in range(2)]
            # v with ones row: [s_tile part, 16 s-tiles, 4 heads, 64+1]
            v_sb = vsb_pool.tile([128, 16, 4, DH + 1], BF16, tag="v")
            nc.vector.memset(
                v_sb[:].rearrange("p a b c -> p (a b c)"), 1.0)

            def rope_big(qb, dst):
                # qb [128, S] holds bias-added projection; swap halves via DMA
                qbs = qbs_pool.tile([128, S], F32, tag="qbs")
                for h in (0, 1):
                    p0 = h * 64
                    nc.sync.dma_start(out=qbs[p0:p0 + 32, :],
                                      in_=qb[p0 + 32:p0 + 64, :])
                    nc.sync.dma_start(out=qbs[p0 + 32:p0 + 64, :],
                                      in_=qb[p0:p0 + 32, :])
                nc.vector.tensor_mul(out=dst[:], in0=qb[:], in1=cos2_sb[:])
                tmp = tmp_pool.tile([128, S], F32R, tag="tmp")
                nc.vector.tensor_mul(out=tmp[:], in0=qbs[:], in1=sinx_sb[:])
                nc.gpsimd.tensor_add(out=dst[:], in0=dst[:], in1=tmp[:])

            # ---- phase A: projections (proj-outer) + RoPE + v transpose ----
            for proj in range(3):
                w_sb = (wq_sb, wk_sb, wv_sb)[proj]
                b_sb = b_sbs[proj]
                for pt in range(2):
                    qb = None
                    if proj < 2:
                        qb = qb_pool.tile([128, S], F32, tag="qb")
                    for st in range(NST):
                        ss = slice(st * 512, (st + 1) * 512)
                        ps = ps_pool.tile([128, 512], F32, tag="ps")
                        for kt in range(8):
                            nc.tensor.matmul(
                                ps[:], w_sb[:, kt, pt * 128:(pt + 1) * 128],
                                x_all[:, kt, ss],
                                start=(kt == 0), stop=(kt == 7))
                        if proj < 2:
                            nc.scalar.activation(
                                out=qb[:, ss], in_=ps[:],
                                func=mybir.ActivationFunctionType.Identity,
                                bias=b_sb[:, pt:pt + 1])
                        else:
                            vt = vtmp_pool.tile([128, 512], F32, tag="vt")
                            nc.scalar.activation(
                                out=vt[:], in_=ps[:],
                                func=mybir.ActivationFunctionType.Identity,
                                bias=bv_sb[:, pt:pt + 1])
                            for j in range(4):
                                sti = st * 4 + j
                                ps_tr = big_pool.tile([128, 128], F32,
                                                      tag="big")
                                nc.tensor.transpose(
                                    ps_tr[:], vt[:, j * 128:(j + 1) * 128],
                                    ident[:])
                                nc.vector.tensor_copy(
                                    out=v_sb[:, sti, 2 * pt:2 * pt + 2, 0:DH],
                                    in_=ps_tr[:].rearrange(
                                        "p (h d) -> p h d", h=2))
                    if proj == 0:
                        rope_big(qb, qrot[pt])
                    elif proj == 1:
                        rope_big(qb, krot[pt])

            # ---- phase B: attention (qt-outer; per-qt Wo partial +
            # chunked bf16 ReduceScatter overlapping later q tiles) ----
            woin = [woin_pool.tile([128, S], F32R, tag="woin", name=f"woin{i}")
                    for i in range(2)]
            def scores(kt, qt, pair, qsl):
                ksl = slice(kt * 128, (kt + 1) * 128)
                ps_s = big_pool.tile([128, 1024], F32, tag="big",
                                     name="ps_s")
                nc.tensor.matmul(
                    ps_s[:, 0:512], krot[pair][0:64, ksl],
                    qrot[pair][0:64, qsl], start=True, stop=True)
                nc.tensor.matmul(
                    ps_s[:, 512:1024], krot[pair][64:128, ksl],
                    qrot[pair][64:128, qsl], start=True, stop=True)
                pr = probs_pool.tile([128, 1024], BF16, tag="pr", name="pr")
                psv = ps_s[:].rearrange("p (h q) -> p h q", h=2)
                prv = pr[:].rearrange("p (h q) -> p h q", h=2)
                off = kt * 128 - qt * 512
                if off <= 0:
                    nc.scalar.activation(out=pr[:], in_=ps_s[:],
                                         func=EXP, scale=0.125)
                else:
                    nc.scalar.activation(out=prv[:, :, off:512],
                                         in_=psv[:, :, off:512],
                                         func=EXP, scale=0.125)
                if off >= 0:
                    nc.gpsimd.affine_select(
                        out=prv[:, :, off:off + 128],
                        in_=prv[:, :, off:off + 128],
                        pattern=[[0, 2], [1, 128]],
                        compare_op=AOP.is_ge, fill=0.0,
                        base=0,
                        channel_multiplier=-1)
                return pr

            def pv(kt, pr, pv_a, pv_b, h0, nkt, qt):
                off = max(0, kt * 128 - qt * 512)
                nc.tensor.matmul(
                    pv_a[:, off:512], v_sb[:, kt, h0, :],
                    pr[:, off:512],
                    start=(kt == 0), stop=(kt == nkt - 1))
                nc.tensor.matmul(
                    pv_b[:, off:512], v_sb[:, kt, h0 + 1, :],
                    pr[:, 512 + off:1024],
                    start=(kt == 0), stop=(kt == nkt - 1))

            def wo_block(qt):
                qsl = slice(qt * 512, (qt + 1) * 512)
                for dm in range(8):
                    ps_o = ps_pool.tile([128, 512], F32, tag="ps",
                                        name="ps_o")
                    for pt in range(2):
                        nc.tensor.matmul(
                            ps_o[:], wo_sb[:, pt, dm * 128:(dm + 1) * 128],
                            woin[pt][:, qsl], start=(pt == 0), stop=(pt == 1))
                    ot = osb_pool.tile([128, QT], F32, tag="ot")
                    if dm % 2 == 0:
                        nc.vector.tensor_scalar_add(out=ot[:], in0=ps_o[:],
                                                    scalar1=bo_sb[:, dm:dm + 1])
                    else:
                        nc.scalar.activation(
                            out=ot[:], in_=ps_o[:],
                            func=mybir.ActivationFunctionType.Identity,
                            bias=bo_sb[:, dm:dm + 1])
                    nc.sync.dma_start(
                        out=out_e[dm * 128:(dm + 1) * 128, qsl], in_=ot[:])

            for qt in range(NQT):
                qsl = slice(qt * 512, (qt + 1) * 512)
                for pair in range(2):
                    pv_a = ps_pool.tile([DH + 1, 512], F32, tag="ps",
                                        name="pv_a")
                    pv_b = ps_pool.tile([DH + 1, 512], F32, tag="ps",
                                        name="pv_b")
                    nkt = 4 * qt + 4
                    h0 = 2 * pair

                    # software pipeline: scores run one kt ahead of pv
                    pr_prev = scores(0, qt, pair, qsl)
                    for kt in range(1, nkt):
                        pr_k = scores(kt, qt, pair, qsl)
                        pv(kt - 1, pr_prev, pv_a, pv_b, h0, nkt, qt)
                        pr_prev = pr_k
                    pv(nkt - 1, pr_prev, pv_a, pv_b, h0, nkt, qt)

                    rec = rec_pool.tile([1, 1024], F32, tag="rec")
                    nc.vector.reciprocal(out=rec[0:1, 0:512],
                                         in_=pv_a[DH:DH + 1, :])
                    nc.vector.reciprocal(out=rec[0:1, 512:1024],
                                         in_=pv_b[DH:DH + 1, :])
                    rb = rb_pool.tile([128, 1024], F32, tag="rb")
                    nc.gpsimd.partition_broadcast(rb[:], rec[0:1, :])
                    nc.vector.tensor_mul(out=woin[pair][0:64, qsl],
                                         in0=pv_a[0:DH, :],
                                         in1=rb[0:64, 0:512])
                    nc.vector.tensor_mul(out=woin[pair][64:128, qsl],
                                         in0=pv_b[0:DH, :],
                                         in1=rb[64:128, 512:1024])
                if qt > 0:
                    wo_block(qt - 1)
            wo_block(NQT - 1)


    nc.compile()
    return nc


def kernel(qkv, cos, sin, Wq, bq, Wk, bk, Wv, bv, Wo, bo):
    from concourse.bass_utils import run_bass_kernel_spmd

    qkv = np.asarray(qkv, dtype=np.float32)
    cos = np.asarray(cos, dtype=np.float32)
    sin = np.asarray(sin, dtype=np.float32)
    Wq, bq = np.asarray(Wq, np.float32), np.asarray(bq, np.float32)
    Wk, bk = np.asarray(Wk, np.float32), np.asarray(bk, np.float32)
    Wv, bv = np.asarray(Wv, np.float32), np.asarray(bv, np.float32)
    Wo, bo = np.asarray(Wo, np.float32), np.asarray(bo, np.float32)

    if "nc" not in _cache:
        _cache["nc"] = _build()
    nc = _cache["nc"]

    cos2 = np.ascontiguousarray(np.tile(cos.T, (2, 1)))  # [128, S]
    sinx = np.tile(sin.T, (2, 1)).copy()
    sinx[0:32] *= -1.0
    sinx[64:96] *= -1.0
    sinx = np.ascontiguousarray(sinx)

    bf = ml_dtypes.bfloat16
    bo4 = np.ascontiguousarray(bo * 0.25)
    in_maps = []
    for c in range(N_CORES):
        b, g = c // 4, c % 4
        hsl = slice(g * FPC, (g + 1) * FPC)
        in_maps.append({
            "qkvT": np.ascontiguousarray(qkv[b].T.astype(bf)),
            "wq": np.ascontiguousarray(Wq[hsl, :].T.astype(bf)),
            "wk": np.ascontiguousarray(Wk[hsl, :].T.astype(bf)),
            "wv": np.ascontiguousarray(Wv[hsl, :].T.astype(bf)),
            "wo": np.ascontiguousarray(Wo[:, hsl].T),
            "bq": np.ascontiguousarray(bq[hsl]),
            "bk": np.ascontiguousarray(bk[hsl]),
            "bv": np.ascontiguousarray(bv[hsl]),
            "bo": bo4,
            "cos2": cos2,
            "sinx": sinx,
        })

    trace = bool(os.environ.get("KERNEL_TRACE"))
    res = run_bass_kernel_spmd(nc, in_maps, list(range(N_CORES)), trace=trace)
    last_run_info["exec_time_ns"] = res.exec_time_ns
    last_run_info["results"] = res

    out = np.empty((B, S, D), dtype=np.float32)
    for b in range(B):
        oT = (res.results[4 * b]["out"] + res.results[4 * b + 1]["out"]
              + res.results[4 * b + 2]["out"] + res.results[4 * b + 3]["out"])
        out[b] = oT.T
    return out
